# revision 24
# baseline (speedup 1.0000x reference)
"""Trainium2 Bass kernel for nn_AttnFusion (scatter_memory).

Strategy
--------
The dense (n=4, 512*512, 128) BEV grid is never materialized.  At a spatial
location the tiny 4-way attention only involves the <=4 points that scatter
there, and the outputs only read locations that have points.  With
w_p = exp(<x0_g, x_p>/sqrt(d)) (x0_g = the cav-0 feature of p's location, or 0)
the ego-query attention reduces to, per unique location g:

    ctx_g       = sum_p w_p x_p / (sum_p w_p + 4 - cnt_g)
    mean_coor_g = sum_p coor_p / cnt_g

(the "4 - cnt" term accounts for the empty cav slots, whose score is 0).

Sharding (spatial-bucketed point partitioning, per the problem hint): sort
points by (spatial, cav), split the sorted unique locations into 8 contiguous
spatial buckets balanced by point count - one bucket per NeuronCore.  Each
core receives its bucket's points already packed into 128-point tiles
(groups never straddle a tile), i.e. the host does the shard + bucket
layout; the attention math, the cav-0 key gather and the per-location
segment reduction run on device.

Per-core device program over tiles of 128 points:
  * "light" tiles hold single-point locations with no cav-0 point: their
    attention degenerates to ctx = x/4, mean = own coor.
  * "heavy" tiles hold everything else:
      1. stream A (point rows); dma_gather B (the matching cav-0 rows from
         the same table; a shared zero row when absent).
      2. s = rowsum(A*B); w = exp(s/sqrt(d))               (DVE + ACT)
      3. per tile a weighted one-hot lhsT[p,g] = w_p*(gid_p==g) in one fused
         DVE op, then two PE matmuls segment-sum [A | m, m/w, c1/w, c2/w]
         into PSUM -> per-group [numer | sum_w | cnt | sum_c1 | sum_c2].
      4. finalize (ACT divides via per-partition scale),
  * results stream out linearly in tile layout; the host unshard step
    compacts (tile, slot) rows back to the global sorted-unique order and
    applies the jnp.unique-style padding.
Inputs that break any packing budget fall back to a numpy implementation.
"""
import math

import numpy as np

# ---------------------------------------------------------------- constants
N = 200_000
D = 128
NCAV = 4
XY = 512 * 512
N_CORES = 8
P = 128

L_TILES = 64               # light tiles (single-point, non-cav0 locations)
H_TILES = 144              # heavy tiles
N_TILES = L_TILES + H_TILES        # 208
P_MAX = N_TILES * P                # 26624 padded points per core
ZROW = P_MAX                       # zero row of the feature table
TRASH = P_MAX                      # invalid marker in the dest map
TBL_ROWS = P_MAX + 1

SSB_TILES = 16             # tiles per superblock (2048 rows)
BLK_TILES = 4              # heavy tiles per PSUM block
N_SSB = N_TILES // SSB_TILES       # 13
L_SSB = L_TILES // SSB_TILES       # 4
H_SSB = H_TILES // SSB_TILES       # 9
SSB_IDX = SSB_TILES * P // 16      # 128 int16 cols per idx tile

SCALE = 1.0 / math.sqrt(float(D))

_PROG = None               # compiled Bass program (built once per process)


# ------------------------------------------------------------ device program
def _build_program(bufs=None, inner_loop=1, all_light=False, blk_tiles=None):
    bufs = bufs or {}
    import concourse.bacc as bacc
    import concourse.tile as tile
    from concourse import mybir
    from contextlib import ExitStack, nullcontext

    BT = blk_tiles or BLK_TILES
    f32 = mybir.dt.float32
    i16 = mybir.dt.int16
    Alu = mybir.AluOpType
    Act = mybir.ActivationFunctionType

    nc = bacc.Bacc("TRN2", target_bir_lowering=False, debug=False,
                   enable_asserts=False, num_devices=N_CORES)

    feat_tbl = nc.dram_tensor("feat_tbl", [N_SSB, 128, SSB_TILES, D], f32,
                              kind="ExternalInput").ap()
    b_tbl = nc.dram_tensor("b_tbl", [H_SSB, 128, SSB_TILES, D], f32,
                           kind="ExternalInput").ap()
    auxf = nc.dram_tensor("auxf", [N_SSB, 128, SSB_TILES * 4], f32,
                          kind="ExternalInput").ap()
    iota_in = nc.dram_tensor("iota_in", [128, 128], f32,
                             kind="ExternalInput").ap()
    out_lin = nc.dram_tensor("out_lin", [N_SSB, 128, SSB_TILES, D], f32,
                             kind="ExternalOutput").ap()
    mc_tab = nc.dram_tensor("mc_tab", [N_SSB, 128, SSB_TILES * 2], f32,
                            kind="ExternalOutput").ap()

    NIDX = SSB_TILES * P           # 2048 rows per superblock

    with tile.TileContext(nc) as tc:
        with ExitStack() as ctx:
            const_p = ctx.enter_context(tc.tile_pool(name="const", bufs=1))
            gbuf_p = ctx.enter_context(
                tc.tile_pool(name="gbuf", bufs=bufs.get("gbuf", 5)))
            idx_p = ctx.enter_context(
                tc.tile_pool(name="idx", bufs=bufs.get("idx", 4)))
            aux_p = ctx.enter_context(
                tc.tile_pool(name="aux", bufs=bufs.get("aux", 6)))
            work_p = ctx.enter_context(
                tc.tile_pool(name="work", bufs=bufs.get("work", 6)))
            oh_p = ctx.enter_context(
                tc.tile_pool(name="oh", bufs=bufs.get("oh", 12)))
            f_p = ctx.enter_context(
                tc.tile_pool(name="fout", bufs=bufs.get("fout", 5)))
            prod_p = ctx.enter_context(
                tc.tile_pool(name="prod", bufs=bufs.get("prod", 3)))
            psum_p = ctx.enter_context(
                tc.tile_pool(name="psum", bufs=bufs.get("psum", 4),
                             space="PSUM"))

            iota_t = const_p.tile([128, 128], f32)
            nc.sync.dma_start(out=iota_t[:], in_=iota_in[:])

            loop_cm = (tc.For_i(0, inner_loop, 1) if inner_loop > 1
                       else nullcontext())
            with loop_cm:
              _order = [0, 4, 5, 1, 6, 7, 2, 8, 9, 3, 10, 11, 12]
              for ssb in _order:
                light = all_light or ssb < L_SSB

                a_sb = gbuf_p.tile([128, SSB_TILES, D], f32, tag="a_sb")
                nc.sync.dma_start(out=a_sb[:], in_=feat_tbl[ssb])

                aux_t = aux_p.tile([128, SSB_TILES, 4], f32, tag="aux_t")
                nc.sync.dma_start(
                    out=aux_t[:],
                    in_=auxf[ssb].rearrange("p (t c) -> p t c", c=4))

                f_t = f_p.tile([128, SSB_TILES, D], f32, tag="f_t")
                mc_sb = aux_p.tile([128, SSB_TILES, 2], f32, tag="mc_sb")

                if light:
                    # ctx = A/4, mean = own coor
                    nc.vector.tensor_scalar_mul(f_t[:], a_sb[:], 0.25)
                    nc.vector.tensor_copy(
                        out=mc_sb[:], in_=aux_t[:, :, 2:4])
                else:
                    hb = ssb - L_SSB
                    b_sb = gbuf_p.tile([128, SSB_TILES, D], f32, tag="b_sb")
                    nc.sync.dma_start(out=b_sb[:], in_=b_tbl[hb])

                    # per-SSB batched: dot products, exp, 1/w, meta columns
                    prod = prod_p.tile([128, SSB_TILES, D], f32, tag="prod")
                    nc.vector.tensor_tensor(
                        out=prod[:], in0=a_sb[:], in1=b_sb[:], op=Alu.mult)
                    s16 = work_p.tile([128, SSB_TILES], f32, tag="s16")
                    nc.vector.tensor_reduce(
                        out=s16[:], in_=prod[:],
                        axis=mybir.AxisListType.X, op=Alu.add)
                    w16 = work_p.tile([128, SSB_TILES], f32, tag="w16")
                    nc.scalar.activation(
                        out=w16[:], in_=s16[:], func=Act.Exp, scale=SCALE)
                    rw16 = work_p.tile([128, SSB_TILES], f32, tag="rw16")
                    nc.vector.reciprocal(out=rw16[:], in_=w16[:])
                    # M = [m, m/w, c1/w, c2/w]
                    m16 = work_p.tile([128, SSB_TILES, 4], f32, tag="m16")
                    nc.vector.tensor_copy(
                        out=m16[:, :, 0:1], in_=aux_t[:, :, 1:2])
                    nc.vector.tensor_tensor(
                        out=m16[:, :, 1:4], in0=aux_t[:, :, 1:4],
                        in1=rw16[:, :, None].to_broadcast(
                            [128, SSB_TILES, 3]),
                        op=Alu.mult)

                    for blk in range(SSB_TILES // BT):
                        c0 = blk * BT
                        a4 = a_sb[:, c0:c0 + BT, :]
                        x4 = aux_t[:, c0:c0 + BT, :]
                        w4 = w16[:, c0:c0 + BT]
                        m4 = m16[:, c0:c0 + BT, :]

                        psum_t = psum_p.tile([128, BT, 256], f32)
                        for t in range(BT):
                            oh = oh_p.tile([128, 128], f32)
                            nc.vector.tensor_scalar(
                                out=oh[:], in0=iota_t[:],
                                scalar1=x4[:, t:t + 1, 0:1],
                                scalar2=w4[:, t:t + 1],
                                op0=Alu.is_equal, op1=Alu.mult)
                            nc.tensor.matmul(
                                out=psum_t[:, t:t + 1, 0:D], lhsT=oh[:],
                                rhs=a4[:, t:t + 1, :], start=True, stop=True)
                            nc.tensor.matmul(
                                out=psum_t[:, t:t + 1, D:D + 4], lhsT=oh[:],
                                rhs=m4[:, t:t + 1, :], start=True, stop=True)

                        # dc = [den | cntc]; cnt clamped to >=1 (real groups
                        # have cnt>=1; empty rows have numer=0 so the
                        # denominator shift is moot)
                        dc = work_p.tile([128, 2, BT], f32, tag="dc")
                        nc.vector.tensor_scalar_max(
                            dc[:, 1, :], psum_t[:, :, D + 1:D + 2], 1.0)
                        nc.vector.scalar_tensor_tensor(
                            out=dc[:, 0, :], in0=psum_t[:, :, D:D + 1],
                            scalar=4.0, in1=dc[:, 1, :],
                            op0=Alu.add, op1=Alu.subtract)
                        rdc = work_p.tile([128, 2, BT], f32, tag="rdc")
                        nc.vector.reciprocal(out=rdc[:], in_=dc[:])
                        rden = rdc[:, 0, :]
                        rcnt = rdc[:, 1, :]

                        for t in range(BT):
                            ft = c0 + t
                            nc.scalar.activation(
                                out=f_t[:, ft:ft + 1, 0:D],
                                in_=psum_t[:, t:t + 1, 0:D],
                                func=Act.Copy, scale=rden[:, t:t + 1])
                        nc.vector.tensor_tensor(
                            out=mc_sb[:, c0:c0 + BT, :],
                            in0=psum_t[:, :, D + 2:D + 4],
                            in1=rcnt[:, :, None].to_broadcast(
                                [128, BT, 2]),
                            op=Alu.mult)

                nc.scalar.dma_start(
                    out=mc_tab[ssb],
                    in_=mc_sb[:].rearrange("p t c -> p (t c)"))
                nc.scalar.dma_start(out=out_lin[ssb], in_=f_t[:])
    nc.compile()
    return nc


def _get_program():
    global _PROG
    if _PROG is None:
        _PROG = _build_program()
    return _PROG


# ------------------------------------------------------------- host helpers
def _wrap_idx(a):
    """Per-row indices [n] -> wrapped [128, n/16] int16 (8 replicas of 16)."""
    w = np.ascontiguousarray(a.reshape(-1, 16).T).astype(np.int16)
    return np.tile(w, (8, 1))


def _host_preprocess(feat, coor, flat_idx):
    """Build the 8 per-core input maps.  Returns None if the input violates
    a packing budget (caller falls back to numpy)."""
    flat = flat_idx.astype(np.int64)
    if flat.shape != (N,) or np.any(np.diff(flat) < 0):
        return None
    if np.any(np.diff(flat) == 0) or flat[0] < 0 or flat[-1] >= NCAV * XY:
        return None
    spatial = flat % XY
    cav = flat // XY
    order = np.argsort(spatial * NCAV + cav)
    s_spatial = spatial[order]
    s_cav = cav[order]

    newgrp = np.empty(N, dtype=bool)
    newgrp[0] = True
    newgrp[1:] = s_spatial[1:] != s_spatial[:-1]
    gstarts = np.flatnonzero(newgrp)
    U = len(gstarts)
    gid_of_point = np.cumsum(newgrp) - 1
    gsizes = np.diff(np.append(gstarts, N))

    has_cav0 = s_cav[gstarts] == 0

    targets = (np.arange(1, N_CORES) * N) // N_CORES
    split_g = np.searchsorted(gstarts, targets, side="left")
    g_bounds = np.concatenate([[0], split_g, [U]])
    p_bounds = np.concatenate([[0], gstarts[split_g], [N]])
    if np.any(np.diff(g_bounds) <= 0):
        return None

    coor12 = np.ascontiguousarray(coor[:, 1:3]).astype(np.float32)
    iota = np.tile(np.arange(128, dtype=np.float32), (128, 1))

    in_maps = []
    meta = []
    for c in range(N_CORES):
        g0, g1 = int(g_bounds[c]), int(g_bounds[c + 1])
        p0, p1 = int(p_bounds[c]), int(p_bounds[c + 1])
        Pc, Gc = p1 - p0, g1 - g0

        gsz = gsizes[g0:g1]                       # [Gc]
        # light groups: single point, not cav0
        is_light = (gsz == 1) & (~has_cav0[g0:g1])
        light_g = np.flatnonzero(is_light)
        n_light = min(len(light_g), L_TILES * P)
        light_g_used = light_g[:n_light]
        heavy_mask = np.ones(Gc, dtype=bool)
        heavy_mask[light_g_used] = False
        heavy_g = np.flatnonzero(heavy_mask)

        # slot assignment: light tiles sequential, heavy tiles greedy-packed
        slot_of_point = np.empty(Pc, dtype=np.int64)   # sorted-point -> slot
        lt = light_g_used
        lslot = np.arange(n_light)
        # light groups are single points: sorted point index = gstarts[g0+lt]
        slot_of_point[gstarts[g0 + lt] - p0] = lslot

        hsz = gsz[heavy_g]
        cum = np.concatenate([[0], np.cumsum(hsz)])
        nH = len(heavy_g)
        tile_first = []
        g = 0
        while g < nH:
            tile_first.append(g)
            jb = int(np.searchsorted(cum, cum[g] + P, side="right")) - 1
            if jb <= g:
                return None
            g = jb
        tile_first = np.asarray(tile_first, dtype=np.int64)
        if len(tile_first) > H_TILES:
            return None
        tile_end = np.append(tile_first[1:], nH)
        gcount = tile_end - tile_first
        if len(gcount) and gcount.max() > P:
            return None

        tile_of_h = np.repeat(np.arange(len(tile_first)), gcount)   # [nH]
        lgid_of_h = np.arange(nH) - tile_first[tile_of_h]
        slot_in_tile_h = cum[:-1] - cum[tile_first[tile_of_h]]

        hg_of_group = np.full(Gc, -1, dtype=np.int64)
        hg_of_group[heavy_g] = np.arange(nH)
        pg = gid_of_point[p0:p1] - g0             # group of each point [Pc]
        hsel = hg_of_group[pg] >= 0               # heavy points
        hp_g = hg_of_group[pg[hsel]]
        pr = (np.arange(p0, p1) - gstarts[g0 + pg])[hsel]
        slot_of_point[hsel] = (L_TILES * P + tile_of_h[hp_g] * P
                               + slot_in_tile_h[hp_g] + pr)

        # --- padded, bucketed feature table (the per-core shard layout) ---
        feat_flat = np.zeros((P_MAX + 128, D), dtype=np.float32)
        oi = order[p0:p1]
        feat_flat[slot_of_point] = feat[oi]

        # cav-0 key slot per group (in table coords), ZROW when absent
        src0_slot = np.full(Gc, ZROW, dtype=np.int64)
        c0sel = has_cav0[g0:g1]
        src0_slot[c0sel] = slot_of_point[gstarts[g0:g1][c0sel] - p0]

        aux = np.zeros((N_TILES, P, 4), dtype=np.float32)
        dest = np.full((N_TILES, P), TRASH, dtype=np.int64)
        b_idx = np.full((H_TILES, P), ZROW, dtype=np.int64)

        l_tile = lslot // P
        l_p = lslot % P
        aux[l_tile, l_p, 2:4] = coor12[oi[gstarts[g0 + lt] - p0]]
        dest[l_tile, l_p] = lt

        ht = tile_of_h[hp_g]
        hslot = slot_in_tile_h[hp_g] + pr
        aux[L_TILES + ht, hslot, 0] = lgid_of_h[hp_g]
        aux[L_TILES + ht, hslot, 1] = 1.0
        aux[L_TILES + ht, hslot, 2:4] = coor12[oi[hsel]]
        b_idx[ht, hslot] = src0_slot[pg[hsel]]
        dest[L_TILES + tile_of_h, lgid_of_h] = heavy_g

        feat_dev = np.ascontiguousarray(
            feat_flat[:P_MAX].reshape(N_SSB, SSB_TILES, P, D)
            .transpose(0, 2, 1, 3))
        b_dev = np.ascontiguousarray(
            feat_flat[b_idx.reshape(-1)].reshape(H_SSB, SSB_TILES, P, D)
            .transpose(0, 2, 1, 3))
        aux_dev = np.ascontiguousarray(
            aux.reshape(N_SSB, SSB_TILES, P, 4).transpose(0, 2, 1, 3)
            .reshape(N_SSB, P, SSB_TILES * 4))

        in_maps.append({
            "feat_tbl": feat_dev, "b_tbl": b_dev,
            "auxf": aux_dev, "iota_in": iota,
        })
        meta.append((g0, Gc, dest))
    return in_maps, meta, U, s_spatial[-1] == XY - 1


# --------------------------------------------------------- numpy fallback
def _numpy_fallback(feat, coor, flat_idx, n):
    n = int(n)
    feat = np.asarray(feat, dtype=np.float32)
    coor = np.asarray(coor, dtype=np.float32)
    flat = np.asarray(flat_idx).astype(np.int64)
    spatial = flat % XY
    cav = flat // XY
    order = np.argsort(spatial * (int(cav.max()) + 1) + cav, kind="stable")
    s_sp = spatial[order]
    newgrp = np.empty(len(flat), dtype=bool)
    newgrp[0] = True
    newgrp[1:] = s_sp[1:] != s_sp[:-1]
    gstarts = np.flatnonzero(newgrp)
    U = len(gstarts)
    gid = np.cumsum(newgrp) - 1
    s_cav = cav[order]
    has0 = s_cav[gstarts] == 0
    x0 = np.zeros((U, feat.shape[1]), np.float32)
    x0[has0] = feat[order[gstarts[has0]]]
    xp = feat[order]
    s = (xp * x0[gid]).sum(1) * np.float32(1.0 / math.sqrt(feat.shape[1]))
    w = np.exp(s)
    numer = np.add.reduceat(xp * w[:, None], gstarts, axis=0)
    sw = np.add.reduceat(w, gstarts)
    cnt = np.diff(np.append(gstarts, len(flat))).astype(np.float32)
    den = sw + (n - cnt)
    ctx = numer / den[:, None]
    c12 = np.add.reduceat(coor[order][:, 1:3].astype(np.float32), gstarts,
                          axis=0)
    mc = c12 / np.maximum(cnt, 1.0)[:, None]

    NN = len(flat)
    fused = np.zeros((NN, feat.shape[1]), np.float32)
    mean = np.zeros((NN, 3), np.float32)
    fused[:U] = ctx
    mean[:U, 1:3] = mc
    if U < NN and (spatial == XY - 1).any():
        fused[U:] = fused[U - 1]
    return fused, mean


# ------------------------------------------------------------------- kernel
def kernel(feat, coor, flat_idx, n):
    feat = np.ascontiguousarray(np.asarray(feat, dtype=np.float32))
    coor = np.ascontiguousarray(np.asarray(coor, dtype=np.float32))
    flat_idx_in = np.asarray(flat_idx)
    if (int(n) != NCAV or feat.shape != (N, D) or coor.shape != (N, 3)
            or flat_idx_in.shape != (N,)):
        return _numpy_fallback(feat, coor, flat_idx_in, n)

    pre = _host_preprocess(feat, coor, flat_idx_in)
    if pre is None:
        return _numpy_fallback(feat, coor, flat_idx_in, n)
    in_maps, meta, U, last_is_corner = pre

    from concourse.bass_utils import run_bass_kernel_spmd
    nc = _get_program()
    res = run_bass_kernel_spmd(nc, in_maps, core_ids=list(range(N_CORES)))

    fused = np.zeros((N, D), np.float32)
    mean = np.zeros((N, 3), np.float32)
    for c in range(N_CORES):
        g0, Gc, dest = meta[c]
        lin = (res.results[c]["out_lin"]
               .reshape(N_SSB, P, SSB_TILES, D)
               .transpose(0, 2, 1, 3).reshape(N_TILES, P, D))
        valid = dest != TRASH
        mc_core = np.zeros((Gc, 2), np.float32)
        m2 = (res.results[c]["mc_tab"]
              .reshape(N_SSB, 128, SSB_TILES, 2)
              .transpose(0, 2, 1, 3).reshape(N_TILES, P, 2))
        dv = dest[valid]
        fused[g0:g0 + Gc][dv] = lin[valid]
        mc_core[dv] = m2[valid]
        mean[g0:g0 + Gc, 1:3] = mc_core
    if U < N and last_is_corner:
        fused[U:] = fused[U - 1]
    return fused, mean


# revision 25
# speedup vs baseline: 1.4424x; 1.4424x over previous
"""Trainium2 Bass kernel for nn_AttnFusion (scatter_memory).

Strategy
--------
The dense (n=4, 512*512, 128) BEV grid is never materialized.  At a spatial
location the tiny 4-way attention only involves the <=4 points that scatter
there, and the outputs only read locations that have points.  With
w_p = exp(<x0_g, x_p>/sqrt(d)) (x0_g = the cav-0 feature of p's location, or 0)
the ego-query attention reduces to, per unique location g:

    ctx_g       = sum_p w_p x_p / (sum_p w_p + 4 - cnt_g)
    mean_coor_g = sum_p coor_p / cnt_g

(the "4 - cnt" term accounts for the empty cav slots, whose score is 0).

Sharding (spatial-bucketed point partitioning, per the problem hint): sort
points by (spatial, cav), split the sorted unique locations into 8 contiguous
spatial buckets balanced by point count - one bucket per NeuronCore.  Each
core receives its bucket's points already packed into 128-point tiles
(groups never straddle a tile), i.e. the host does the shard + bucket
layout; the attention math, the cav-0 key gather and the per-location
segment reduction run on device.

Per-core device program over tiles of 128 points:
  * "light" tiles hold single-point locations with no cav-0 point: their
    attention degenerates to ctx = x/4, mean = own coor.
  * "heavy" tiles hold everything else:
      1. stream A (point rows); dma_gather B (the matching cav-0 rows from
         the same table; a shared zero row when absent).
      2. s = rowsum(A*B); w = exp(s/sqrt(d))               (DVE + ACT)
      3. per tile a weighted one-hot lhsT[p,g] = w_p*(gid_p==g) in one fused
         DVE op, then two PE matmuls segment-sum [A | m, m/w, c1/w, c2/w]
         into PSUM -> per-group [numer | sum_w | cnt | sum_c1 | sum_c2].
      4. finalize (ACT divides via per-partition scale),
  * results stream out linearly in tile layout; the host unshard step
    compacts (tile, slot) rows back to the global sorted-unique order and
    applies the jnp.unique-style padding.
Inputs that break any packing budget fall back to a numpy implementation.
"""
import math

import numpy as np

# ---------------------------------------------------------------- constants
N = 200_000
D = 128
NCAV = 4
XY = 512 * 512
N_CORES = 8
P = 128

L_TILES = 64               # light tiles (single-point, non-cav0 locations)
H_TILES = 144              # heavy tiles
N_TILES = L_TILES + H_TILES        # 208
P_MAX = N_TILES * P                # 26624 padded points per core
ZROW = P_MAX                       # zero row of the feature table
TRASH = P_MAX                      # invalid marker in the dest map
TBL_ROWS = P_MAX + 1

SSB_TILES = 16             # tiles per superblock (2048 rows)
BLK_TILES = 4              # heavy tiles per PSUM block
N_SSB = N_TILES // SSB_TILES       # 13
L_SSB = L_TILES // SSB_TILES       # 4
H_SSB = H_TILES // SSB_TILES       # 9
SSB_IDX = SSB_TILES * P // 16      # 128 int16 cols per idx tile

SCALE = 1.0 / math.sqrt(float(D))

_PROG = None               # compiled Bass program (built once per process)


# ------------------------------------------------------------ device program
def _build_program(bufs=None, inner_loop=1, all_light=False, blk_tiles=None):
    bufs = bufs or {}
    import concourse.bacc as bacc
    import concourse.tile as tile
    from concourse import mybir
    from contextlib import ExitStack, nullcontext

    BT = blk_tiles or BLK_TILES
    f32 = mybir.dt.float32
    i16 = mybir.dt.int16
    Alu = mybir.AluOpType
    Act = mybir.ActivationFunctionType

    nc = bacc.Bacc("TRN2", target_bir_lowering=False, debug=False,
                   enable_asserts=False, num_devices=N_CORES)

    feat_tbl = nc.dram_tensor("feat_tbl", [N_SSB, 128, SSB_TILES, D], f32,
                              kind="ExternalInput").ap()
    b_tbl = nc.dram_tensor("b_tbl", [H_SSB, 128, SSB_TILES, D], f32,
                           kind="ExternalInput").ap()
    auxf = nc.dram_tensor("auxf", [N_SSB, 128, SSB_TILES * 4], f32,
                          kind="ExternalInput").ap()
    iota_in = nc.dram_tensor("iota_in", [128, 128], f32,
                             kind="ExternalInput").ap()
    out_lin = nc.dram_tensor("out_lin", [N_SSB, 128, SSB_TILES, D], f32,
                             kind="ExternalOutput").ap()
    mc_tab = nc.dram_tensor("mc_tab", [N_SSB, 128, SSB_TILES * 2], f32,
                            kind="ExternalOutput").ap()

    NIDX = SSB_TILES * P           # 2048 rows per superblock

    with tile.TileContext(nc) as tc:
        with ExitStack() as ctx:
            const_p = ctx.enter_context(tc.tile_pool(name="const", bufs=1))
            gbuf_p = ctx.enter_context(
                tc.tile_pool(name="gbuf", bufs=bufs.get("gbuf", 5)))
            idx_p = ctx.enter_context(
                tc.tile_pool(name="idx", bufs=bufs.get("idx", 4)))
            aux_p = ctx.enter_context(
                tc.tile_pool(name="aux", bufs=bufs.get("aux", 6)))
            work_p = ctx.enter_context(
                tc.tile_pool(name="work", bufs=bufs.get("work", 6)))
            oh_p = ctx.enter_context(
                tc.tile_pool(name="oh", bufs=bufs.get("oh", 12)))
            f_p = ctx.enter_context(
                tc.tile_pool(name="fout", bufs=bufs.get("fout", 5)))
            prod_p = ctx.enter_context(
                tc.tile_pool(name="prod", bufs=bufs.get("prod", 3)))
            psum_p = ctx.enter_context(
                tc.tile_pool(name="psum", bufs=bufs.get("psum", 4),
                             space="PSUM"))

            iota_t = const_p.tile([128, 128], f32)
            nc.sync.dma_start(out=iota_t[:], in_=iota_in[:])

            loop_cm = (tc.For_i(0, inner_loop, 1) if inner_loop > 1
                       else nullcontext())
            with loop_cm:
              for ssb in range(N_SSB):
                light = all_light or ssb < L_SSB

                a_sb = gbuf_p.tile([128, SSB_TILES, D], f32, tag="a_sb")
                nc.sync.dma_start(out=a_sb[:], in_=feat_tbl[ssb])

                aux_t = aux_p.tile([128, SSB_TILES, 4], f32, tag="aux_t")
                nc.sync.dma_start(
                    out=aux_t[:],
                    in_=auxf[ssb].rearrange("p (t c) -> p t c", c=4))

                f_t = f_p.tile([128, SSB_TILES, D], f32, tag="f_t")
                mc_sb = aux_p.tile([128, SSB_TILES, 2], f32, tag="mc_sb")

                if light:
                    # ctx = A/4, mean = own coor
                    nc.vector.tensor_scalar_mul(f_t[:], a_sb[:], 0.25)
                    nc.vector.tensor_copy(
                        out=mc_sb[:], in_=aux_t[:, :, 2:4])
                else:
                    hb = ssb - L_SSB
                    b_sb = gbuf_p.tile([128, SSB_TILES, D], f32, tag="b_sb")
                    nc.sync.dma_start(out=b_sb[:], in_=b_tbl[hb])

                    # per-SSB batched: dot products, exp, 1/w, meta columns
                    prod = prod_p.tile([128, SSB_TILES, D], f32, tag="prod")
                    nc.vector.tensor_tensor(
                        out=prod[:], in0=a_sb[:], in1=b_sb[:], op=Alu.mult)
                    s16 = work_p.tile([128, SSB_TILES], f32, tag="s16")
                    nc.vector.tensor_reduce(
                        out=s16[:], in_=prod[:],
                        axis=mybir.AxisListType.X, op=Alu.add)
                    w16 = work_p.tile([128, SSB_TILES], f32, tag="w16")
                    nc.scalar.activation(
                        out=w16[:], in_=s16[:], func=Act.Exp, scale=SCALE)
                    rw16 = work_p.tile([128, SSB_TILES], f32, tag="rw16")
                    nc.vector.reciprocal(out=rw16[:], in_=w16[:])
                    # M = [m, m/w, c1/w, c2/w]
                    m16 = work_p.tile([128, SSB_TILES, 4], f32, tag="m16")
                    nc.vector.tensor_copy(
                        out=m16[:, :, 0:1], in_=aux_t[:, :, 1:2])
                    nc.vector.tensor_tensor(
                        out=m16[:, :, 1:4], in0=aux_t[:, :, 1:4],
                        in1=rw16[:, :, None].to_broadcast(
                            [128, SSB_TILES, 3]),
                        op=Alu.mult)

                    for blk in range(SSB_TILES // BT):
                        c0 = blk * BT
                        a4 = a_sb[:, c0:c0 + BT, :]
                        x4 = aux_t[:, c0:c0 + BT, :]
                        w4 = w16[:, c0:c0 + BT]
                        m4 = m16[:, c0:c0 + BT, :]

                        psum_t = psum_p.tile([128, BT, 256], f32)
                        for t in range(BT):
                            oh = oh_p.tile([128, 128], f32)
                            nc.vector.tensor_scalar(
                                out=oh[:], in0=iota_t[:],
                                scalar1=x4[:, t:t + 1, 0:1],
                                scalar2=w4[:, t:t + 1],
                                op0=Alu.is_equal, op1=Alu.mult)
                            nc.tensor.matmul(
                                out=psum_t[:, t:t + 1, 0:D], lhsT=oh[:],
                                rhs=a4[:, t:t + 1, :], start=True, stop=True)
                            nc.tensor.matmul(
                                out=psum_t[:, t:t + 1, D:D + 4], lhsT=oh[:],
                                rhs=m4[:, t:t + 1, :], start=True, stop=True)

                        # dc = [den | cntc]; cnt clamped to >=1 (real groups
                        # have cnt>=1; empty rows have numer=0 so the
                        # denominator shift is moot)
                        dc = work_p.tile([128, 2, BT], f32, tag="dc")
                        nc.vector.tensor_scalar_max(
                            dc[:, 1, :], psum_t[:, :, D + 1:D + 2], 1.0)
                        nc.vector.scalar_tensor_tensor(
                            out=dc[:, 0, :], in0=psum_t[:, :, D:D + 1],
                            scalar=4.0, in1=dc[:, 1, :],
                            op0=Alu.add, op1=Alu.subtract)
                        rdc = work_p.tile([128, 2, BT], f32, tag="rdc")
                        nc.vector.reciprocal(out=rdc[:], in_=dc[:])
                        rden = rdc[:, 0, :]
                        rcnt = rdc[:, 1, :]

                        for t in range(BT):
                            ft = c0 + t
                            nc.scalar.activation(
                                out=f_t[:, ft:ft + 1, 0:D],
                                in_=psum_t[:, t:t + 1, 0:D],
                                func=Act.Copy, scale=rden[:, t:t + 1])
                        nc.vector.tensor_tensor(
                            out=mc_sb[:, c0:c0 + BT, :],
                            in0=psum_t[:, :, D + 2:D + 4],
                            in1=rcnt[:, :, None].to_broadcast(
                                [128, BT, 2]),
                            op=Alu.mult)

                nc.scalar.dma_start(
                    out=mc_tab[ssb],
                    in_=mc_sb[:].rearrange("p t c -> p (t c)"))
                nc.scalar.dma_start(out=out_lin[ssb], in_=f_t[:])
    nc.compile()
    return nc


def _get_program():
    global _PROG
    if _PROG is None:
        _PROG = _build_program()
    return _PROG


# ------------------------------------------------------------- host helpers
def _wrap_idx(a):
    """Per-row indices [n] -> wrapped [128, n/16] int16 (8 replicas of 16)."""
    w = np.ascontiguousarray(a.reshape(-1, 16).T).astype(np.int16)
    return np.tile(w, (8, 1))


def _host_preprocess(feat, coor, flat_idx):
    """Build the 8 per-core input maps.  Returns None if the input violates
    a packing budget (caller falls back to numpy)."""
    flat = flat_idx.astype(np.int64)
    if flat.shape != (N,) or np.any(np.diff(flat) < 0):
        return None
    if np.any(np.diff(flat) == 0) or flat[0] < 0 or flat[-1] >= NCAV * XY:
        return None
    spatial = flat % XY
    cav = flat // XY
    order = np.argsort(spatial * NCAV + cav)
    s_spatial = spatial[order]
    s_cav = cav[order]

    newgrp = np.empty(N, dtype=bool)
    newgrp[0] = True
    newgrp[1:] = s_spatial[1:] != s_spatial[:-1]
    gstarts = np.flatnonzero(newgrp)
    U = len(gstarts)
    gid_of_point = np.cumsum(newgrp) - 1
    gsizes = np.diff(np.append(gstarts, N))

    has_cav0 = s_cav[gstarts] == 0

    targets = (np.arange(1, N_CORES) * N) // N_CORES
    split_g = np.searchsorted(gstarts, targets, side="left")
    g_bounds = np.concatenate([[0], split_g, [U]])
    p_bounds = np.concatenate([[0], gstarts[split_g], [N]])
    if np.any(np.diff(g_bounds) <= 0):
        return None

    coor12 = np.ascontiguousarray(coor[:, 1:3]).astype(np.float32)
    iota = np.tile(np.arange(128, dtype=np.float32), (128, 1))

    in_maps = []
    meta = []
    for c in range(N_CORES):
        g0, g1 = int(g_bounds[c]), int(g_bounds[c + 1])
        p0, p1 = int(p_bounds[c]), int(p_bounds[c + 1])
        Pc, Gc = p1 - p0, g1 - g0

        gsz = gsizes[g0:g1]                       # [Gc]
        # light groups: single point, not cav0
        is_light = (gsz == 1) & (~has_cav0[g0:g1])
        light_g = np.flatnonzero(is_light)
        n_light = min(len(light_g), L_TILES * P)
        light_g_used = light_g[:n_light]
        heavy_mask = np.ones(Gc, dtype=bool)
        heavy_mask[light_g_used] = False
        heavy_g = np.flatnonzero(heavy_mask)

        # slot assignment: light tiles sequential, heavy tiles greedy-packed
        slot_of_point = np.empty(Pc, dtype=np.int64)   # sorted-point -> slot
        lt = light_g_used
        lslot = np.arange(n_light)
        # light groups are single points: sorted point index = gstarts[g0+lt]
        slot_of_point[gstarts[g0 + lt] - p0] = lslot

        hsz = gsz[heavy_g]
        cum = np.concatenate([[0], np.cumsum(hsz)])
        nH = len(heavy_g)
        tile_first = []
        g = 0
        while g < nH:
            tile_first.append(g)
            jb = int(np.searchsorted(cum, cum[g] + P, side="right")) - 1
            if jb <= g:
                return None
            g = jb
        tile_first = np.asarray(tile_first, dtype=np.int64)
        if len(tile_first) > H_TILES:
            return None
        tile_end = np.append(tile_first[1:], nH)
        gcount = tile_end - tile_first
        if len(gcount) and gcount.max() > P:
            return None

        tile_of_h = np.repeat(np.arange(len(tile_first)), gcount)   # [nH]
        lgid_of_h = np.arange(nH) - tile_first[tile_of_h]
        slot_in_tile_h = cum[:-1] - cum[tile_first[tile_of_h]]

        hg_of_group = np.full(Gc, -1, dtype=np.int64)
        hg_of_group[heavy_g] = np.arange(nH)
        pg = gid_of_point[p0:p1] - g0             # group of each point [Pc]
        hsel = hg_of_group[pg] >= 0               # heavy points
        hp_g = hg_of_group[pg[hsel]]
        pr = (np.arange(p0, p1) - gstarts[g0 + pg])[hsel]
        slot_of_point[hsel] = (L_TILES * P + tile_of_h[hp_g] * P
                               + slot_in_tile_h[hp_g] + pr)

        # --- padded, bucketed feature table (the per-core shard layout) ---
        feat_flat = np.zeros((P_MAX + 128, D), dtype=np.float32)
        oi = order[p0:p1]
        feat_flat[slot_of_point] = feat[oi]

        # cav-0 key slot per group (in table coords), ZROW when absent
        src0_slot = np.full(Gc, ZROW, dtype=np.int64)
        c0sel = has_cav0[g0:g1]
        src0_slot[c0sel] = slot_of_point[gstarts[g0:g1][c0sel] - p0]

        aux = np.zeros((N_TILES, P, 4), dtype=np.float32)
        dest = np.full((N_TILES, P), TRASH, dtype=np.int64)
        b_idx = np.full((H_TILES, P), ZROW, dtype=np.int64)

        l_tile = lslot // P
        l_p = lslot % P
        aux[l_tile, l_p, 2:4] = coor12[oi[gstarts[g0 + lt] - p0]]
        dest[l_tile, l_p] = lt

        ht = tile_of_h[hp_g]
        hslot = slot_in_tile_h[hp_g] + pr
        aux[L_TILES + ht, hslot, 0] = lgid_of_h[hp_g]
        aux[L_TILES + ht, hslot, 1] = 1.0
        aux[L_TILES + ht, hslot, 2:4] = coor12[oi[hsel]]
        b_idx[ht, hslot] = src0_slot[pg[hsel]]
        dest[L_TILES + tile_of_h, lgid_of_h] = heavy_g

        feat_dev = np.ascontiguousarray(
            feat_flat[:P_MAX].reshape(N_SSB, SSB_TILES, P, D)
            .transpose(0, 2, 1, 3))
        b_dev = np.ascontiguousarray(
            feat_flat[b_idx.reshape(-1)].reshape(H_SSB, SSB_TILES, P, D)
            .transpose(0, 2, 1, 3))
        aux_dev = np.ascontiguousarray(
            aux.reshape(N_SSB, SSB_TILES, P, 4).transpose(0, 2, 1, 3)
            .reshape(N_SSB, P, SSB_TILES * 4))

        in_maps.append({
            "feat_tbl": feat_dev, "b_tbl": b_dev,
            "auxf": aux_dev, "iota_in": iota,
        })
        meta.append((g0, Gc, dest))
    return in_maps, meta, U, s_spatial[-1] == XY - 1


# --------------------------------------------------------- numpy fallback
def _numpy_fallback(feat, coor, flat_idx, n):
    n = int(n)
    feat = np.asarray(feat, dtype=np.float32)
    coor = np.asarray(coor, dtype=np.float32)
    flat = np.asarray(flat_idx).astype(np.int64)
    spatial = flat % XY
    cav = flat // XY
    order = np.argsort(spatial * (int(cav.max()) + 1) + cav, kind="stable")
    s_sp = spatial[order]
    newgrp = np.empty(len(flat), dtype=bool)
    newgrp[0] = True
    newgrp[1:] = s_sp[1:] != s_sp[:-1]
    gstarts = np.flatnonzero(newgrp)
    U = len(gstarts)
    gid = np.cumsum(newgrp) - 1
    s_cav = cav[order]
    has0 = s_cav[gstarts] == 0
    x0 = np.zeros((U, feat.shape[1]), np.float32)
    x0[has0] = feat[order[gstarts[has0]]]
    xp = feat[order]
    s = (xp * x0[gid]).sum(1) * np.float32(1.0 / math.sqrt(feat.shape[1]))
    w = np.exp(s)
    numer = np.add.reduceat(xp * w[:, None], gstarts, axis=0)
    sw = np.add.reduceat(w, gstarts)
    cnt = np.diff(np.append(gstarts, len(flat))).astype(np.float32)
    den = sw + (n - cnt)
    ctx = numer / den[:, None]
    c12 = np.add.reduceat(coor[order][:, 1:3].astype(np.float32), gstarts,
                          axis=0)
    mc = c12 / np.maximum(cnt, 1.0)[:, None]

    NN = len(flat)
    fused = np.zeros((NN, feat.shape[1]), np.float32)
    mean = np.zeros((NN, 3), np.float32)
    fused[:U] = ctx
    mean[:U, 1:3] = mc
    if U < NN and (spatial == XY - 1).any():
        fused[U:] = fused[U - 1]
    return fused, mean


# ------------------------------------------------------------------- kernel
def kernel(feat, coor, flat_idx, n):
    feat = np.ascontiguousarray(np.asarray(feat, dtype=np.float32))
    coor = np.ascontiguousarray(np.asarray(coor, dtype=np.float32))
    flat_idx_in = np.asarray(flat_idx)
    if (int(n) != NCAV or feat.shape != (N, D) or coor.shape != (N, 3)
            or flat_idx_in.shape != (N,)):
        return _numpy_fallback(feat, coor, flat_idx_in, n)

    pre = _host_preprocess(feat, coor, flat_idx_in)
    if pre is None:
        return _numpy_fallback(feat, coor, flat_idx_in, n)
    in_maps, meta, U, last_is_corner = pre

    from concourse.bass_utils import run_bass_kernel_spmd
    nc = _get_program()
    res = run_bass_kernel_spmd(nc, in_maps, core_ids=list(range(N_CORES)))

    fused = np.zeros((N, D), np.float32)
    mean = np.zeros((N, 3), np.float32)
    for c in range(N_CORES):
        g0, Gc, dest = meta[c]
        lin = (res.results[c]["out_lin"]
               .reshape(N_SSB, P, SSB_TILES, D)
               .transpose(0, 2, 1, 3).reshape(N_TILES, P, D))
        valid = dest != TRASH
        mc_core = np.zeros((Gc, 2), np.float32)
        m2 = (res.results[c]["mc_tab"]
              .reshape(N_SSB, 128, SSB_TILES, 2)
              .transpose(0, 2, 1, 3).reshape(N_TILES, P, 2))
        dv = dest[valid]
        fused[g0:g0 + Gc][dv] = lin[valid]
        mc_core[dv] = m2[valid]
        mean[g0:g0 + Gc, 1:3] = mc_core
    if U < N and last_is_corner:
        fused[U:] = fused[U - 1]
    return fused, mean
